# revision 6
# baseline (speedup 1.0000x reference)
"""Cross-attention Trainium2 kernel, 8-core SPMD.

Problem: B=2, L=S=2048, E=1024, H=16 heads of D=64.
  q = x@Wq+bq; k = ctx@Wk+bk; v = ctx@Wv+bv  (per-head split)
  out = softmax(q k^T / sqrt(D)) v, heads concat, @Wp + bp

Sharding: DP over batch (2) x TP over heads (4 groups of 4 heads).
Core c: batch b=c//4, head group g=c%4 (heads 4g..4g+3).
Each core computes a partial projection output [L, E]; the host sums the 4
partials per batch and adds bp (Megatron-style TP reduce done host-side).

On-device dataflow (per core), everything transposed so no on-chip
transposes are ever needed:
  QT[dh, L] = Wq_c.T @ xT       (xT, ctxT fed pre-transposed from host)
  KT[dh, S] = Wk_c.T @ ctxT
  V[S, dh]  = (ctxT tiles).T @ Wv_c   (+ ones column per head for row-sums)
  per head pair (row-tiled K=64 matmul pair), per 512-col chunk of L:
    scoresT[s_tile, lc] = KT_tile.T @ QT_chunk        (PSUM [128,1024], 2 heads)
    PT = exp(0.125 * scoresT)                          (one ACT op, fp32r out)
    OT'[65, lc] += (V|1).T @ PT                        (accumulated over s)
  normalize: sums row broadcast via ones-outer-product matmul, DVE recip+mul
  outP[L, E] partial = OT.T @ Wp_c                     (PSUM accum over dh)

All matmul operands are float32r (TF32-like, ~1.5e-4 matmul rel err, full
PE rate at free-dim>=256); PSUM accumulation is fp32.
"""
import sys

if "/opt/trn_rl_repo" not in sys.path:
    sys.path.insert(0, "/opt/trn_rl_repo")

import numpy as np

import concourse.bacc as bacc
import concourse.tile as tile
import concourse.mybir as mybir
from concourse.bass_utils import run_bass_kernel_spmd

F32 = mybir.dt.float32
F32R = mybir.dt.float32r
EXP = mybir.ActivationFunctionType.Exp
ADD = mybir.AluOpType.add
MULT = mybir.AluOpType.mult

B, L, S, E, H, D = 2, 2048, 2048, 1024, 16, 64
NCORES = 8
TPG = 4          # tensor-parallel group size (head groups)
DH = E // TPG    # per-core head dims = 256 (4 heads)
NE = E // 128    # 8 contraction tiles
NC5 = 512        # column chunk
NLC = L // NC5   # 4 L-chunks
NST = S // 128   # 16 S tiles
SCALE = 1.0 / np.sqrt(np.float32(D))

_cache = {}


def _build():
    nc = bacc.Bacc("TRN2", target_bir_lowering=False, debug=False, num_devices=1)
    xT = nc.dram_tensor("xT", [E, L], F32R, kind="ExternalInput").ap()
    ctxT = nc.dram_tensor("ctxT", [E, S], F32R, kind="ExternalInput").ap()
    wq = nc.dram_tensor("wq", [E, DH], F32R, kind="ExternalInput").ap()
    wk = nc.dram_tensor("wk", [E, DH], F32R, kind="ExternalInput").ap()
    wv = nc.dram_tensor("wv", [E, DH], F32R, kind="ExternalInput").ap()
    wp = nc.dram_tensor("wp", [DH, E], F32R, kind="ExternalInput").ap()
    bq = nc.dram_tensor("bq", [DH, 1], F32, kind="ExternalInput").ap()
    bk = nc.dram_tensor("bk", [DH, 1], F32, kind="ExternalInput").ap()
    bv = nc.dram_tensor("bv", [1, DH], F32R, kind="ExternalInput").ap()
    onesr = nc.dram_tensor("onesr", [1, 128], F32R, kind="ExternalInput").ap()
    onesc = nc.dram_tensor("onesc", [128, 1], F32R, kind="ExternalInput").ap()
    outp = nc.dram_tensor("outp", [L, E], F32, kind="ExternalOutput").ap()

    with tile.TileContext(nc) as tc:
        with (
            tc.tile_pool(name="wgt", bufs=1) as wgt,
            tc.tile_pool(name="small", bufs=1) as small,
            tc.tile_pool(name="stream", bufs=2) as stream,
            tc.tile_pool(name="qt", bufs=2) as qtp,
            tc.tile_pool(name="kt", bufs=2) as ktp,
            tc.tile_pool(name="vt", bufs=NST) as vtp,
            tc.tile_pool(name="pt", bufs=3) as ptp,
            tc.tile_pool(name="ot", bufs=2) as otp,
            tc.tile_pool(name="ob", bufs=2) as obp,
            tc.tile_pool(name="nrm", bufs=2) as nrm,
            tc.tile_pool(name="gemm", bufs=2, space="PSUM") as gemm,
            tc.tile_pool(name="sc", bufs=2, space="PSUM") as scp,
            tc.tile_pool(name="av", bufs=2, space="PSUM") as avp,
        ):
            # ---- weights / biases / constants ----
            wq_sb = [wgt.tile([128, DH], F32R, tag=f"wq{e}", name=f"wq{e}") for e in range(NE)]
            wk_sb = [wgt.tile([128, DH], F32R, tag=f"wk{e}", name=f"wk{e}") for e in range(NE)]
            wv_sb = [wgt.tile([128, DH], F32R, tag=f"wv{e}", name=f"wv{e}") for e in range(NE)]
            for e in range(NE):
                nc.sync.dma_start(wq_sb[e][:], wq[e * 128:(e + 1) * 128, :])
                nc.sync.dma_start(wk_sb[e][:], wk[e * 128:(e + 1) * 128, :])
                nc.sync.dma_start(wv_sb[e][:], wv[e * 128:(e + 1) * 128, :])
            wp_sb = [wgt.tile([128, E], F32R, tag=f"wp{m}", name=f"wp{m}") for m in range(2)]
            for m in range(2):
                nc.sync.dma_start(wp_sb[m][:], wp[m * 128:(m + 1) * 128, :])
            bq_sb = [small.tile([128, 1], F32, tag=f"bq{m}", name=f"bq{m}") for m in range(2)]
            bk_sb = [small.tile([128, 1], F32, tag=f"bk{m}", name=f"bk{m}") for m in range(2)]
            for m in range(2):
                nc.sync.dma_start(bq_sb[m][:], bq[m * 128:(m + 1) * 128, :])
                nc.sync.dma_start(bk_sb[m][:], bk[m * 128:(m + 1) * 128, :])
            bv_sb = small.tile([1, DH], F32R, tag="bv")
            nc.sync.dma_start(bv_sb[:], bv[:])
            ones = small.tile([1, 128], F32R, tag="ones")
            nc.sync.dma_start(ones[:], onesr[:])
            ones_col = small.tile([128, 1], F32R, tag="ones_col")
            nc.sync.dma_start(ones_col[:], onesc[:])

            # chunked [E, 512] views of xT/ctxT: SBUF [128, NE*512], e-major
            def chunk_ap(src, c):
                v = src[:, c * NC5:(c + 1) * NC5]
                return v.rearrange("(e p) l -> p e l", p=128)

            # ---- QT = Wq.T @ xT, per L-chunk ----
            qt_sb = [qtp.tile([128, L], F32R, tag=f"qt{m}", name=f"qt{m}") for m in range(2)]
            for c in range(NLC):
                xc = stream.tile([128, NE * NC5], F32R, tag="stream")
                nc.sync.dma_start(xc[:].rearrange("p (e l) -> p e l", e=NE), chunk_ap(xT, c))
                for m in range(2):
                    pq = gemm.tile([128, NC5], F32, tag="gemm")
                    for e in range(NE):
                        nc.tensor.matmul(
                            pq[:],
                            wq_sb[e][:, m * 128:(m + 1) * 128],
                            xc[:, e * NC5:(e + 1) * NC5],
                            start=(e == 0), stop=(e == NE - 1),
                        )
                    nc.vector.tensor_scalar(
                        qt_sb[m][:, c * NC5:(c + 1) * NC5], pq[:],
                        bq_sb[m][:, 0:1], None, ADD,
                    )

            # ---- KT / V from ctxT, per S-chunk ----
            kt_sb = [ktp.tile([128, S], F32R, tag=f"kt{m}", name=f"kt{m}") for m in range(2)]
            v_sb = [vtp.tile([128, 4 * 65], F32R, tag="vt", name=f"vt{s}") for s in range(NST)]
            for c in range(NLC):
                cc = stream.tile([128, NE * NC5], F32R, tag="stream")
                nc.sync.dma_start(cc[:].rearrange("p (e l) -> p e l", e=NE), chunk_ap(ctxT, c))
                for m in range(2):
                    pk = gemm.tile([128, NC5], F32, tag="gemm")
                    for e in range(NE):
                        nc.tensor.matmul(
                            pk[:],
                            wk_sb[e][:, m * 128:(m + 1) * 128],
                            cc[:, e * NC5:(e + 1) * NC5],
                            start=(e == 0), stop=(e == NE - 1),
                        )
                    nc.vector.tensor_scalar(
                        kt_sb[m][:, c * NC5:(c + 1) * NC5], pk[:],
                        bk_sb[m][:, 0:1], None, ADD,
                    )
                for si in range(4):
                    s = c * 4 + si
                    pv = gemm.tile([128, DH], F32, tag="gemm")
                    for e in range(NE):
                        nc.tensor.matmul(
                            pv[:],
                            cc[:, e * NC5 + si * 128: e * NC5 + (si + 1) * 128],
                            wv_sb[e][:],
                            start=(e == 0), stop=False,
                        )
                    nc.tensor.matmul(
                        pv[:], ones[0:1, :], bv_sb[:], start=False, stop=True,
                    )
                    vt = v_sb[s]
                    for h in range(4):
                        nc.vector.tensor_copy(
                            vt[:, h * 65:h * 65 + 64],
                            pv[:, h * 64:(h + 1) * 64],
                        )
                        nc.vector.tensor_copy(
                            vt[:, h * 65 + 64:h * 65 + 65], ones_col[:],
                        )

            # ---- attention + projection, per L-chunk ----
            ot_sb = [otp.tile([128, L], F32R, tag=f"ot{m}", name=f"ot{m}") for m in range(2)]
            for c in range(NLC):
                for hp in range(2):
                    av = [avp.tile([65, NC5], F32, tag="av", name=f"av{j}") for j in range(2)]
                    for s in range(NST):
                        sc = scp.tile([128, 2 * NC5], F32, tag="sc")
                        for j in range(2):
                            nc.tensor.matmul(
                                sc[:, j * NC5:(j + 1) * NC5],
                                kt_sb[hp][j * 64:(j + 1) * 64,
                                          s * 128:(s + 1) * 128],
                                qt_sb[hp][j * 64:(j + 1) * 64,
                                          c * NC5:(c + 1) * NC5],
                                start=True, stop=True,
                            )
                        pt = ptp.tile([128, 2 * NC5], F32R, tag="pt")
                        nc.scalar.activation(pt[:], sc[:], EXP, scale=float(SCALE))
                        for j in range(2):
                            nc.tensor.matmul(
                                av[j][:],
                                v_sb[s][:, (hp * 2 + j) * 65:(hp * 2 + j + 1) * 65],
                                pt[:, j * NC5:(j + 1) * NC5],
                                start=(s == 0), stop=(s == NST - 1),
                            )
                    for j in range(2):
                        sums = nrm.tile([1, NC5], F32R, tag="sums")
                        nc.vector.tensor_copy(sums[:], av[j][64:65, :])
                        rbp = gemm.tile([64, NC5], F32, tag="gemm")
                        nc.tensor.matmul(
                            rbp[:], ones[0:1, 0:64], sums[:], start=True, stop=True,
                        )
                        rb = nrm.tile([64, NC5], F32, tag="rb")
                        nc.vector.reciprocal(rb[:], rbp[:])
                        nc.vector.tensor_tensor(
                            ot_sb[hp][j * 64:(j + 1) * 64, c * NC5:(c + 1) * NC5],
                            av[j][0:64, :], rb[:], MULT,
                        )
                # projection for this L-chunk
                for i in range(4):
                    lt = c * 4 + i
                    ob = obp.tile([128, E], F32, tag="ob")
                    for nch in range(2):
                        pp = gemm.tile([128, NC5], F32, tag="gemm")
                        for m in range(2):
                            nc.tensor.matmul(
                                pp[:],
                                ot_sb[m][:, lt * 128:(lt + 1) * 128],
                                wp_sb[m][:, nch * NC5:(nch + 1) * NC5],
                                start=(m == 0), stop=(m == 1),
                            )
                        nc.vector.tensor_copy(
                            ob[:, nch * NC5:(nch + 1) * NC5], pp[:],
                        )
                    nc.sync.dma_start(outp[lt * 128:(lt + 1) * 128, :], ob[:])

    nc.compile()
    return nc


def _get_nc():
    if "nc" not in _cache:
        _cache["nc"] = _build()
    return _cache["nc"]


def kernel(x, context, Wq, bq, Wk, bk, Wv, bv, Wp, bp):
    x = np.asarray(x, dtype=np.float32)
    context = np.asarray(context, dtype=np.float32)
    Wq, Wk, Wv, Wp = (np.asarray(a, dtype=np.float32) for a in (Wq, Wk, Wv, Wp))
    bq, bk, bv, bp = (np.asarray(a, dtype=np.float32) for a in (bq, bk, bv, bp))

    nc = _get_nc()
    in_maps = []
    for c in range(NCORES):
        b, g = divmod(c, TPG)
        sl = slice(g * DH, (g + 1) * DH)
        in_maps.append({
            "xT": np.ascontiguousarray(x[b].T),
            "ctxT": np.ascontiguousarray(context[b].T),
            "wq": np.ascontiguousarray(Wq[:, sl]),
            "wk": np.ascontiguousarray(Wk[:, sl]),
            "wv": np.ascontiguousarray(Wv[:, sl]),
            "wp": np.ascontiguousarray(Wp[sl, :]),
            "bq": np.ascontiguousarray(bq[sl].reshape(DH, 1)),
            "bk": np.ascontiguousarray(bk[sl].reshape(DH, 1)),
            "bv": np.ascontiguousarray(bv[sl].reshape(1, DH)),
            "onesr": np.ones((1, 128), dtype=np.float32),
            "onesc": np.ones((128, 1), dtype=np.float32),
        })

    trace = bool(int(__import__("os").environ.get("KERNEL_TRACE", "0")))
    res = run_bass_kernel_spmd(nc, in_maps, list(range(NCORES)), trace=trace)
    _cache["last_results"] = res

    out = np.zeros((B, L, E), dtype=np.float32)
    for c in range(NCORES):
        b = c // TPG
        out[b] += res.results[c]["outp"]
    out += bp.reshape(1, 1, E)
    return out


# revision 7
# speedup vs baseline: 1.1220x; 1.1220x over previous
"""Cross-attention Trainium2 kernel, 8-core SPMD.

Problem: B=2, L=S=2048, E=1024, H=16 heads of D=64.
  q = x@Wq+bq; k = ctx@Wk+bk; v = ctx@Wv+bv  (per-head split)
  out = softmax(q k^T / sqrt(D)) v, heads concat, @Wp + bp

Sharding: DP over batch (2) x TP over heads (4 groups of 4 heads).
Core c: batch b=c//4, head group g=c%4 (heads 4g..4g+3).
Each core computes a partial projection output [L, E]; the host sums the 4
partials per batch and adds bp (Megatron-style TP reduce done host-side).

On-device dataflow (per core), everything transposed so no on-chip
transposes are ever needed:
  QT[dh, L] = Wq_c.T @ xT       (xT, ctxT fed pre-transposed from host)
  KT[dh, S] = Wk_c.T @ ctxT
  V[S, dh]  = (ctxT tiles).T @ Wv_c   (+ ones column per head for row-sums)
  per head pair (row-tiled K=64 matmul pair), per 512-col chunk of L:
    scoresT[s_tile, lc] = KT_tile.T @ QT_chunk        (PSUM [128,1024], 2 heads)
    PT = exp(0.125 * scoresT)                          (one ACT op, fp32r out)
    OT'[65, lc] += (V|1).T @ PT                        (accumulated over s)
  normalize: sums row broadcast via ones-outer-product matmul, DVE recip+mul
  outP[L, E] partial = OT.T @ Wp_c                     (PSUM accum over dh)

All matmul operands are float16 (full 1-cycle/row PE rate; fp32r lowers to
2-cycle/row fp32-HIGH mode on this compiler); PSUM accumulation is fp32.
"""
import sys

if "/opt/trn_rl_repo" not in sys.path:
    sys.path.insert(0, "/opt/trn_rl_repo")

import numpy as np

import concourse.bacc as bacc
import concourse.tile as tile
import concourse.mybir as mybir
from concourse.bass_utils import run_bass_kernel_spmd

F32 = mybir.dt.float32
F16 = mybir.dt.float16
EXP = mybir.ActivationFunctionType.Exp
ADD = mybir.AluOpType.add
MULT = mybir.AluOpType.mult

B, L, S, E, H, D = 2, 2048, 2048, 1024, 16, 64
NCORES = 8
TPG = 4          # tensor-parallel group size (head groups)
DH = E // TPG    # per-core head dims = 256 (4 heads)
NE = E // 128    # 8 contraction tiles
NC5 = 512        # column chunk
NLC = L // NC5   # 4 L-chunks
NST = S // 128   # 16 S tiles
SCALE = 1.0 / np.sqrt(np.float32(D))

_cache = {}


def _build():
    nc = bacc.Bacc("TRN2", target_bir_lowering=False, debug=False, num_devices=1)
    xT = nc.dram_tensor("xT", [E, L], F16, kind="ExternalInput").ap()
    ctxT = nc.dram_tensor("ctxT", [E, S], F16, kind="ExternalInput").ap()
    wq = nc.dram_tensor("wq", [E, DH], F16, kind="ExternalInput").ap()
    wk = nc.dram_tensor("wk", [E, DH], F16, kind="ExternalInput").ap()
    wv = nc.dram_tensor("wv", [E, DH], F16, kind="ExternalInput").ap()
    wp = nc.dram_tensor("wp", [DH, E], F16, kind="ExternalInput").ap()
    bq = nc.dram_tensor("bq", [DH, 1], F32, kind="ExternalInput").ap()
    bk = nc.dram_tensor("bk", [DH, 1], F32, kind="ExternalInput").ap()
    bv = nc.dram_tensor("bv", [1, DH], F16, kind="ExternalInput").ap()
    onesr = nc.dram_tensor("onesr", [1, 128], F16, kind="ExternalInput").ap()
    onesc = nc.dram_tensor("onesc", [128, 1], F16, kind="ExternalInput").ap()
    outp = nc.dram_tensor("outp", [L, E], F32, kind="ExternalOutput").ap()

    with tile.TileContext(nc) as tc:
        with (
            tc.tile_pool(name="wgt", bufs=1) as wgt,
            tc.tile_pool(name="small", bufs=1) as small,
            tc.tile_pool(name="stream", bufs=2) as stream,
            tc.tile_pool(name="qt", bufs=2) as qtp,
            tc.tile_pool(name="kt", bufs=2) as ktp,
            tc.tile_pool(name="vt", bufs=NST) as vtp,
            tc.tile_pool(name="pt", bufs=3) as ptp,
            tc.tile_pool(name="ot", bufs=2) as otp,
            tc.tile_pool(name="ob", bufs=2) as obp,
            tc.tile_pool(name="nrm", bufs=2) as nrm,
            tc.tile_pool(name="gemm", bufs=2, space="PSUM") as gemm,
            tc.tile_pool(name="sc", bufs=2, space="PSUM") as scp,
            tc.tile_pool(name="av", bufs=2, space="PSUM") as avp,
        ):
            # ---- weights / biases / constants ----
            wq_sb = [wgt.tile([128, DH], F16, tag=f"wq{e}", name=f"wq{e}") for e in range(NE)]
            wk_sb = [wgt.tile([128, DH], F16, tag=f"wk{e}", name=f"wk{e}") for e in range(NE)]
            wv_sb = [wgt.tile([128, DH], F16, tag=f"wv{e}", name=f"wv{e}") for e in range(NE)]
            for e in range(NE):
                nc.sync.dma_start(wq_sb[e][:], wq[e * 128:(e + 1) * 128, :])
                nc.sync.dma_start(wk_sb[e][:], wk[e * 128:(e + 1) * 128, :])
                nc.sync.dma_start(wv_sb[e][:], wv[e * 128:(e + 1) * 128, :])
            wp_sb = [wgt.tile([128, E], F16, tag=f"wp{m}", name=f"wp{m}") for m in range(2)]
            for m in range(2):
                nc.sync.dma_start(wp_sb[m][:], wp[m * 128:(m + 1) * 128, :])
            bq_sb = [small.tile([128, 1], F32, tag=f"bq{m}", name=f"bq{m}") for m in range(2)]
            bk_sb = [small.tile([128, 1], F32, tag=f"bk{m}", name=f"bk{m}") for m in range(2)]
            for m in range(2):
                nc.sync.dma_start(bq_sb[m][:], bq[m * 128:(m + 1) * 128, :])
                nc.sync.dma_start(bk_sb[m][:], bk[m * 128:(m + 1) * 128, :])
            bv_sb = small.tile([1, DH], F16, tag="bv")
            nc.sync.dma_start(bv_sb[:], bv[:])
            ones = small.tile([1, 128], F16, tag="ones")
            nc.sync.dma_start(ones[:], onesr[:])
            ones_col = small.tile([128, 1], F16, tag="ones_col")
            nc.sync.dma_start(ones_col[:], onesc[:])

            # chunked [E, 512] views of xT/ctxT: SBUF [128, NE*512], e-major
            def chunk_ap(src, c):
                v = src[:, c * NC5:(c + 1) * NC5]
                return v.rearrange("(e p) l -> p e l", p=128)

            # ---- QT = Wq.T @ xT, per L-chunk ----
            qt_sb = [qtp.tile([128, L], F16, tag=f"qt{m}", name=f"qt{m}") for m in range(2)]
            for c in range(NLC):
                xc = stream.tile([128, NE * NC5], F16, tag="stream")
                nc.sync.dma_start(xc[:].rearrange("p (e l) -> p e l", e=NE), chunk_ap(xT, c))
                for m in range(2):
                    pq = gemm.tile([128, NC5], F32, tag="gemm")
                    for e in range(NE):
                        nc.tensor.matmul(
                            pq[:],
                            wq_sb[e][:, m * 128:(m + 1) * 128],
                            xc[:, e * NC5:(e + 1) * NC5],
                            start=(e == 0), stop=(e == NE - 1),
                        )
                    nc.vector.tensor_scalar(
                        qt_sb[m][:, c * NC5:(c + 1) * NC5], pq[:],
                        bq_sb[m][:, 0:1], None, ADD,
                    )

            # ---- KT / V from ctxT, per S-chunk ----
            kt_sb = [ktp.tile([128, S], F16, tag=f"kt{m}", name=f"kt{m}") for m in range(2)]
            v_sb = [vtp.tile([128, 4 * 65], F16, tag="vt", name=f"vt{s}") for s in range(NST)]
            for c in range(NLC):
                cc = stream.tile([128, NE * NC5], F16, tag="stream")
                nc.sync.dma_start(cc[:].rearrange("p (e l) -> p e l", e=NE), chunk_ap(ctxT, c))
                for m in range(2):
                    pk = gemm.tile([128, NC5], F32, tag="gemm")
                    for e in range(NE):
                        nc.tensor.matmul(
                            pk[:],
                            wk_sb[e][:, m * 128:(m + 1) * 128],
                            cc[:, e * NC5:(e + 1) * NC5],
                            start=(e == 0), stop=(e == NE - 1),
                        )
                    nc.vector.tensor_scalar(
                        kt_sb[m][:, c * NC5:(c + 1) * NC5], pk[:],
                        bk_sb[m][:, 0:1], None, ADD,
                    )
                for si in range(4):
                    s = c * 4 + si
                    pv = gemm.tile([128, DH], F32, tag="gemm")
                    for e in range(NE):
                        nc.tensor.matmul(
                            pv[:],
                            cc[:, e * NC5 + si * 128: e * NC5 + (si + 1) * 128],
                            wv_sb[e][:],
                            start=(e == 0), stop=False,
                        )
                    nc.tensor.matmul(
                        pv[:], ones[0:1, :], bv_sb[:], start=False, stop=True,
                    )
                    vt = v_sb[s]
                    for h in range(4):
                        nc.vector.tensor_copy(
                            vt[:, h * 65:h * 65 + 64],
                            pv[:, h * 64:(h + 1) * 64],
                        )
                        nc.vector.tensor_copy(
                            vt[:, h * 65 + 64:h * 65 + 65], ones_col[:],
                        )

            # ---- attention + projection, per L-chunk ----
            ot_sb = [otp.tile([128, L], F16, tag=f"ot{m}", name=f"ot{m}") for m in range(2)]
            for c in range(NLC):
                for hp in range(2):
                    av = [avp.tile([65, NC5], F32, tag="av", name=f"av{j}") for j in range(2)]
                    for s in range(NST):
                        sc = scp.tile([128, 2 * NC5], F32, tag="sc")
                        for j in range(2):
                            nc.tensor.matmul(
                                sc[:, j * NC5:(j + 1) * NC5],
                                kt_sb[hp][j * 64:(j + 1) * 64,
                                          s * 128:(s + 1) * 128],
                                qt_sb[hp][j * 64:(j + 1) * 64,
                                          c * NC5:(c + 1) * NC5],
                                start=True, stop=True,
                            )
                        pt = ptp.tile([128, 2 * NC5], F16, tag="pt")
                        nc.scalar.activation(pt[:], sc[:], EXP, scale=float(SCALE))
                        for j in range(2):
                            nc.tensor.matmul(
                                av[j][:],
                                v_sb[s][:, (hp * 2 + j) * 65:(hp * 2 + j + 1) * 65],
                                pt[:, j * NC5:(j + 1) * NC5],
                                start=(s == 0), stop=(s == NST - 1),
                            )
                    for j in range(2):
                        sums = nrm.tile([1, NC5], F16, tag="sums")
                        nc.vector.tensor_copy(sums[:], av[j][64:65, :])
                        rbp = gemm.tile([64, NC5], F32, tag="gemm")
                        nc.tensor.matmul(
                            rbp[:], ones[0:1, 0:64], sums[:], start=True, stop=True,
                        )
                        rb = nrm.tile([64, NC5], F32, tag="rb")
                        nc.vector.reciprocal(rb[:], rbp[:])
                        nc.vector.tensor_tensor(
                            ot_sb[hp][j * 64:(j + 1) * 64, c * NC5:(c + 1) * NC5],
                            av[j][0:64, :], rb[:], MULT,
                        )
                # projection for this L-chunk
                for i in range(4):
                    lt = c * 4 + i
                    ob = obp.tile([128, E], F32, tag="ob")
                    for nch in range(2):
                        pp = gemm.tile([128, NC5], F32, tag="gemm")
                        for m in range(2):
                            nc.tensor.matmul(
                                pp[:],
                                ot_sb[m][:, lt * 128:(lt + 1) * 128],
                                wp_sb[m][:, nch * NC5:(nch + 1) * NC5],
                                start=(m == 0), stop=(m == 1),
                            )
                        nc.vector.tensor_copy(
                            ob[:, nch * NC5:(nch + 1) * NC5], pp[:],
                        )
                    nc.sync.dma_start(outp[lt * 128:(lt + 1) * 128, :], ob[:])

    nc.compile()
    return nc


def _get_nc():
    if "nc" not in _cache:
        _cache["nc"] = _build()
    return _cache["nc"]


def kernel(x, context, Wq, bq, Wk, bk, Wv, bv, Wp, bp):
    x = np.asarray(x, dtype=np.float32)
    context = np.asarray(context, dtype=np.float32)
    Wq, Wk, Wv, Wp = (np.asarray(a, dtype=np.float32) for a in (Wq, Wk, Wv, Wp))
    bq, bk, bv, bp = (np.asarray(a, dtype=np.float32) for a in (bq, bk, bv, bp))

    nc = _get_nc()
    in_maps = []
    for c in range(NCORES):
        b, g = divmod(c, TPG)
        sl = slice(g * DH, (g + 1) * DH)
        f16 = np.float16
        in_maps.append({
            "xT": np.ascontiguousarray(x[b].T).astype(f16),
            "ctxT": np.ascontiguousarray(context[b].T).astype(f16),
            "wq": np.ascontiguousarray(Wq[:, sl]).astype(f16),
            "wk": np.ascontiguousarray(Wk[:, sl]).astype(f16),
            "wv": np.ascontiguousarray(Wv[:, sl]).astype(f16),
            "wp": np.ascontiguousarray(Wp[sl, :]).astype(f16),
            "bq": np.ascontiguousarray(bq[sl].reshape(DH, 1)),
            "bk": np.ascontiguousarray(bk[sl].reshape(DH, 1)),
            "bv": np.ascontiguousarray(bv[sl].reshape(1, DH)).astype(f16),
            "onesr": np.ones((1, 128), dtype=f16),
            "onesc": np.ones((128, 1), dtype=f16),
        })

    trace = bool(int(__import__("os").environ.get("KERNEL_TRACE", "0")))
    res = run_bass_kernel_spmd(nc, in_maps, list(range(NCORES)), trace=trace)
    _cache["last_results"] = res

    out = np.zeros((B, L, E), dtype=np.float32)
    for c in range(NCORES):
        b = c // TPG
        out[b] += res.results[c]["outp"]
    out += bp.reshape(1, 1, E)
    return out


# revision 8
# speedup vs baseline: 1.2645x; 1.1269x over previous
"""Cross-attention Trainium2 kernel, 8-core SPMD.

Problem: B=2, L=S=2048, E=1024, H=16 heads of D=64.
  q = x@Wq+bq; k = ctx@Wk+bk; v = ctx@Wv+bv  (per-head split)
  out = softmax(q k^T / sqrt(D)) v, heads concat, @Wp + bp

Sharding: DP over batch (2) x TP over heads (4 groups of 4 heads).
Core c: batch b=c//4, head group g=c%4 (heads 4g..4g+3).
Each core computes a partial projection output [L, E]; the host sums the 4
partials per batch and adds bp (Megatron-style TP reduce done host-side).

On-device dataflow (per core), everything transposed so no on-chip
transposes are ever needed:
  QT[dh, L] = Wq_c.T @ xT       (xT, ctxT fed pre-transposed from host)
  KT[dh, S] = Wk_c.T @ ctxT
  V[S, dh]  = (ctxT tiles).T @ Wv_c   (+ ones column per head for row-sums)
  per head pair (row-tiled K=64 matmul pair), per 512-col chunk of L:
    scoresT[s_tile, lc] = KT_tile.T @ QT_chunk        (PSUM [128,1024], 2 heads)
    PT = exp(0.125 * scoresT)                          (one ACT op, fp32r out)
    OT'[65, lc] += (V|1).T @ PT                        (accumulated over s)
  normalize: sums row broadcast via ones-outer-product matmul, DVE recip+mul
  outP[L, E] partial = OT.T @ Wp_c                     (PSUM accum over dh)

All matmul operands are float16 (full 1-cycle/row PE rate; fp32r lowers to
2-cycle/row fp32-HIGH mode on this compiler); PSUM accumulation is fp32.
"""
import sys

if "/opt/trn_rl_repo" not in sys.path:
    sys.path.insert(0, "/opt/trn_rl_repo")

import numpy as np

import concourse.bacc as bacc
import concourse.tile as tile
import concourse.mybir as mybir
from concourse.bass_utils import run_bass_kernel_spmd

F32 = mybir.dt.float32
F16 = mybir.dt.float16
EXP = mybir.ActivationFunctionType.Exp
ADD = mybir.AluOpType.add
MULT = mybir.AluOpType.mult

B, L, S, E, H, D = 2, 2048, 2048, 1024, 16, 64
NCORES = 8
TPG = 4          # tensor-parallel group size (head groups)
DH = E // TPG    # per-core head dims = 256 (4 heads)
NE = E // 128    # 8 contraction tiles
NC5 = 512        # column chunk
NLC = L // NC5   # 4 L-chunks
NST = S // 128   # 16 S tiles
SCALE = 1.0 / np.sqrt(np.float32(D))

_cache = {}


def _build():
    nc = bacc.Bacc("TRN2", target_bir_lowering=False, debug=False, num_devices=1)
    xT = nc.dram_tensor("xT", [E, L], F16, kind="ExternalInput").ap()
    ctxT = nc.dram_tensor("ctxT", [E, S], F16, kind="ExternalInput").ap()
    wq = nc.dram_tensor("wq", [E, DH], F16, kind="ExternalInput").ap()
    wk = nc.dram_tensor("wk", [E, DH], F16, kind="ExternalInput").ap()
    wv = nc.dram_tensor("wv", [E, DH], F16, kind="ExternalInput").ap()
    wp = nc.dram_tensor("wp", [DH, E], F16, kind="ExternalInput").ap()
    bq = nc.dram_tensor("bq", [DH, 1], F32, kind="ExternalInput").ap()
    bk = nc.dram_tensor("bk", [DH, 1], F32, kind="ExternalInput").ap()
    bv = nc.dram_tensor("bv", [1, DH], F16, kind="ExternalInput").ap()
    onesr = nc.dram_tensor("onesr", [1, 128], F16, kind="ExternalInput").ap()
    onesc = nc.dram_tensor("onesc", [128, 1], F16, kind="ExternalInput").ap()
    outp = nc.dram_tensor("outp", [L, E], F32, kind="ExternalOutput").ap()

    with tile.TileContext(nc) as tc:
        with (
            tc.tile_pool(name="wgt", bufs=1) as wgt,
            tc.tile_pool(name="small", bufs=1) as small,
            tc.tile_pool(name="stream", bufs=4) as stream,
            tc.tile_pool(name="qt", bufs=2) as qtp,
            tc.tile_pool(name="kt", bufs=2) as ktp,
            tc.tile_pool(name="vt", bufs=NST) as vtp,
            tc.tile_pool(name="pt", bufs=4) as ptp,
            tc.tile_pool(name="ot", bufs=2) as otp,
            tc.tile_pool(name="ob", bufs=3) as obp,
            tc.tile_pool(name="nrm", bufs=2) as nrm,
            tc.tile_pool(name="gemm", bufs=2, space="PSUM") as gemm,
            tc.tile_pool(name="sc", bufs=2, space="PSUM") as scp,
            tc.tile_pool(name="av", bufs=2, space="PSUM") as avp,
        ):
            # ---- weights / biases / constants ----
            wq_sb = [wgt.tile([128, DH], F16, tag=f"wq{e}", name=f"wq{e}") for e in range(NE)]
            wk_sb = [wgt.tile([128, DH], F16, tag=f"wk{e}", name=f"wk{e}") for e in range(NE)]
            wv_sb = [wgt.tile([128, DH], F16, tag=f"wv{e}", name=f"wv{e}") for e in range(NE)]
            for e in range(NE):
                nc.sync.dma_start(wq_sb[e][:], wq[e * 128:(e + 1) * 128, :])
                nc.sync.dma_start(wk_sb[e][:], wk[e * 128:(e + 1) * 128, :])
                nc.sync.dma_start(wv_sb[e][:], wv[e * 128:(e + 1) * 128, :])
            wp_sb = [wgt.tile([128, E], F16, tag=f"wp{m}", name=f"wp{m}") for m in range(2)]
            for m in range(2):
                nc.sync.dma_start(wp_sb[m][:], wp[m * 128:(m + 1) * 128, :])
            bq_sb = [small.tile([128, 1], F32, tag=f"bq{m}", name=f"bq{m}") for m in range(2)]
            bk_sb = [small.tile([128, 1], F32, tag=f"bk{m}", name=f"bk{m}") for m in range(2)]
            for m in range(2):
                nc.sync.dma_start(bq_sb[m][:], bq[m * 128:(m + 1) * 128, :])
                nc.sync.dma_start(bk_sb[m][:], bk[m * 128:(m + 1) * 128, :])
            bv_sb = small.tile([1, DH], F16, tag="bv")
            nc.sync.dma_start(bv_sb[:], bv[:])
            ones = small.tile([1, 128], F16, tag="ones")
            nc.sync.dma_start(ones[:], onesr[:])
            ones_col = small.tile([128, 1], F16, tag="ones_col")
            nc.sync.dma_start(ones_col[:], onesc[:])

            # chunked [E, 512] views of xT/ctxT: SBUF [128, NE*512], e-major
            def chunk_ap(src, c):
                v = src[:, c * NC5:(c + 1) * NC5]
                return v.rearrange("(e p) l -> p e l", p=128)

            # ---- QT = Wq.T @ xT, per L-chunk ----
            qt_sb = [qtp.tile([128, L], F16, tag=f"qt{m}", name=f"qt{m}") for m in range(2)]

            def qt_chunk(c):
                xc = stream.tile([128, NE * NC5], F16, tag="stream", name=f"xc{c}")
                nc.sync.dma_start(xc[:].rearrange("p (e l) -> p e l", e=NE), chunk_ap(xT, c))
                for m in range(2):
                    pq = gemm.tile([128, NC5], F32, tag="gemm", name=f"pq{c}{m}")
                    for e in range(NE):
                        nc.tensor.matmul(
                            pq[:],
                            wq_sb[e][:, m * 128:(m + 1) * 128],
                            xc[:, e * NC5:(e + 1) * NC5],
                            start=(e == 0), stop=(e == NE - 1),
                        )
                    nc.vector.tensor_scalar(
                        qt_sb[m][:, c * NC5:(c + 1) * NC5], pq[:],
                        bq_sb[m][:, 0:1], None, ADD,
                    )

            # chunk 0 first so attention l-chunk 0 is unblocked the moment
            # KT/V finish; chunks 1-3 are emitted after KT/V and overlap
            # the ACT-bound attention phase.
            qt_chunk(0)

            # ---- KT / V from ctxT, per S-chunk ----
            kt_sb = [ktp.tile([128, S], F16, tag=f"kt{m}", name=f"kt{m}") for m in range(2)]
            v_sb = [vtp.tile([128, 4 * 65], F16, tag="vt", name=f"vt{s}") for s in range(NST)]
            for c in range(NLC):
                cc = stream.tile([128, NE * NC5], F16, tag="stream")
                nc.sync.dma_start(cc[:].rearrange("p (e l) -> p e l", e=NE), chunk_ap(ctxT, c))
                for m in range(2):
                    pk = gemm.tile([128, NC5], F32, tag="gemm")
                    for e in range(NE):
                        nc.tensor.matmul(
                            pk[:],
                            wk_sb[e][:, m * 128:(m + 1) * 128],
                            cc[:, e * NC5:(e + 1) * NC5],
                            start=(e == 0), stop=(e == NE - 1),
                        )
                    nc.vector.tensor_scalar(
                        kt_sb[m][:, c * NC5:(c + 1) * NC5], pk[:],
                        bk_sb[m][:, 0:1], None, ADD,
                    )
                for si in range(4):
                    s = c * 4 + si
                    pv = gemm.tile([128, DH], F32, tag="gemm")
                    for e in range(NE):
                        nc.tensor.matmul(
                            pv[:],
                            cc[:, e * NC5 + si * 128: e * NC5 + (si + 1) * 128],
                            wv_sb[e][:],
                            start=(e == 0), stop=False,
                        )
                    nc.tensor.matmul(
                        pv[:], ones[0:1, :], bv_sb[:], start=False, stop=True,
                    )
                    vt = v_sb[s]
                    for h in range(4):
                        nc.vector.tensor_copy(
                            vt[:, h * 65:h * 65 + 64],
                            pv[:, h * 64:(h + 1) * 64],
                        )
                        nc.vector.tensor_copy(
                            vt[:, h * 65 + 64:h * 65 + 65], ones_col[:],
                        )

            for c in range(1, NLC):
                qt_chunk(c)

            # ---- attention + projection, per L-chunk ----
            ot_sb = [otp.tile([128, L], F16, tag=f"ot{m}", name=f"ot{m}") for m in range(2)]
            for c in range(NLC):
                for hp in range(2):
                    av = [avp.tile([65, NC5], F32, tag="av", name=f"av{j}") for j in range(2)]
                    for s in range(NST):
                        sc = scp.tile([128, 2 * NC5], F32, tag="sc")
                        for j in range(2):
                            nc.tensor.matmul(
                                sc[:, j * NC5:(j + 1) * NC5],
                                kt_sb[hp][j * 64:(j + 1) * 64,
                                          s * 128:(s + 1) * 128],
                                qt_sb[hp][j * 64:(j + 1) * 64,
                                          c * NC5:(c + 1) * NC5],
                                start=True, stop=True,
                            )
                        pt = ptp.tile([128, 2 * NC5], F16, tag="pt")
                        nc.scalar.activation(pt[:], sc[:], EXP, scale=float(SCALE))
                        for j in range(2):
                            nc.tensor.matmul(
                                av[j][:],
                                v_sb[s][:, (hp * 2 + j) * 65:(hp * 2 + j + 1) * 65],
                                pt[:, j * NC5:(j + 1) * NC5],
                                start=(s == 0), stop=(s == NST - 1),
                            )
                    for j in range(2):
                        # drain the AV psum bank to SBUF right away so the
                        # next group's AV accumulation can reuse it; the rest
                        # of the normalization runs off SBUF.
                        ov = nrm.tile([65, NC5], F32, tag="ov")
                        nc.vector.tensor_copy(ov[:], av[j][:])
                        sums = nrm.tile([1, NC5], F16, tag="sums")
                        nc.vector.tensor_copy(sums[:], ov[64:65, :])
                        rbp = gemm.tile([64, NC5], F32, tag="gemm")
                        nc.tensor.matmul(
                            rbp[:], ones[0:1, 0:64], sums[:], start=True, stop=True,
                        )
                        rb = nrm.tile([64, NC5], F32, tag="rb")
                        nc.vector.reciprocal(rb[:], rbp[:])
                        nc.vector.tensor_tensor(
                            ot_sb[hp][j * 64:(j + 1) * 64, c * NC5:(c + 1) * NC5],
                            ov[0:64, :], rb[:], MULT,
                        )
                # projection for this L-chunk
                for i in range(4):
                    lt = c * 4 + i
                    ob = obp.tile([128, E], F32, tag="ob")
                    for nch in range(2):
                        pp = gemm.tile([128, NC5], F32, tag="gemm")
                        for m in range(2):
                            nc.tensor.matmul(
                                pp[:],
                                ot_sb[m][:, lt * 128:(lt + 1) * 128],
                                wp_sb[m][:, nch * NC5:(nch + 1) * NC5],
                                start=(m == 0), stop=(m == 1),
                            )
                        nc.vector.tensor_copy(
                            ob[:, nch * NC5:(nch + 1) * NC5], pp[:],
                        )
                    nc.sync.dma_start(outp[lt * 128:(lt + 1) * 128, :], ob[:])

    nc.compile()
    return nc


def _get_nc():
    if "nc" not in _cache:
        _cache["nc"] = _build()
    return _cache["nc"]


def kernel(x, context, Wq, bq, Wk, bk, Wv, bv, Wp, bp):
    x = np.asarray(x, dtype=np.float32)
    context = np.asarray(context, dtype=np.float32)
    Wq, Wk, Wv, Wp = (np.asarray(a, dtype=np.float32) for a in (Wq, Wk, Wv, Wp))
    bq, bk, bv, bp = (np.asarray(a, dtype=np.float32) for a in (bq, bk, bv, bp))

    nc = _get_nc()
    in_maps = []
    for c in range(NCORES):
        b, g = divmod(c, TPG)
        sl = slice(g * DH, (g + 1) * DH)
        f16 = np.float16
        in_maps.append({
            "xT": np.ascontiguousarray(x[b].T).astype(f16),
            "ctxT": np.ascontiguousarray(context[b].T).astype(f16),
            "wq": np.ascontiguousarray(Wq[:, sl]).astype(f16),
            "wk": np.ascontiguousarray(Wk[:, sl]).astype(f16),
            "wv": np.ascontiguousarray(Wv[:, sl]).astype(f16),
            "wp": np.ascontiguousarray(Wp[sl, :]).astype(f16),
            "bq": np.ascontiguousarray(bq[sl].reshape(DH, 1)),
            "bk": np.ascontiguousarray(bk[sl].reshape(DH, 1)),
            "bv": np.ascontiguousarray(bv[sl].reshape(1, DH)).astype(f16),
            "onesr": np.ones((1, 128), dtype=f16),
            "onesc": np.ones((128, 1), dtype=f16),
        })

    trace = bool(int(__import__("os").environ.get("KERNEL_TRACE", "0")))
    res = run_bass_kernel_spmd(nc, in_maps, list(range(NCORES)), trace=trace)
    _cache["last_results"] = res

    out = np.zeros((B, L, E), dtype=np.float32)
    for c in range(NCORES):
        b = c // TPG
        out[b] += res.results[c]["outp"]
    out += bp.reshape(1, 1, E)
    return out


# revision 9
# speedup vs baseline: 1.4649x; 1.1585x over previous
"""Cross-attention Trainium2 kernel, 8-core SPMD.

Problem: B=2, L=S=2048, E=1024, H=16 heads of D=64.
  q = x@Wq+bq; k = ctx@Wk+bk; v = ctx@Wv+bv  (per-head split)
  out = softmax(q k^T / sqrt(D)) v, heads concat, @Wp + bp

Sharding: DP over batch (2) x TP over heads (4 groups of 4 heads).
Core c: batch b=c//4, head group g=c%4 (heads 4g..4g+3).
Each core computes a partial projection output [L, E]; the host sums the 4
partials per batch and adds bp (Megatron-style TP reduce done host-side).

On-device dataflow (per core), everything transposed so no on-chip
transposes are ever needed:
  QT[dh, L] = Wq_c.T @ xT       (xT, ctxT fed pre-transposed from host)
  KT[dh, S] = Wk_c.T @ ctxT
  V[S, dh]  = (ctxT tiles).T @ Wv_c   (+ ones column per head for row-sums)
  per head pair (row-tiled K=64 matmul pair), per 512-col chunk of L:
    scoresT[s_tile, lc] = KT_tile.T @ QT_chunk        (PSUM [128,1024], 2 heads)
    PT = exp(0.125 * scoresT)                          (one ACT op, fp32r out)
    OT'[65, lc] += (V|1).T @ PT                        (accumulated over s)
  normalize: sums row broadcast via ones-outer-product matmul, DVE recip+mul
  outP[L, E] partial = OT.T @ Wp_c                     (PSUM accum over dh)

All matmul operands are float16 (full 1-cycle/row PE rate; fp32r lowers to
2-cycle/row fp32-HIGH mode on this compiler); PSUM accumulation is fp32.
"""
import sys

if "/opt/trn_rl_repo" not in sys.path:
    sys.path.insert(0, "/opt/trn_rl_repo")

import numpy as np

import concourse.bacc as bacc
import concourse.tile as tile
import concourse.mybir as mybir
from concourse.bass_utils import run_bass_kernel_spmd

F32 = mybir.dt.float32
F16 = mybir.dt.float16
EXP = mybir.ActivationFunctionType.Exp
ADD = mybir.AluOpType.add
MULT = mybir.AluOpType.mult

B, L, S, E, H, D = 2, 2048, 2048, 1024, 16, 64
NCORES = 8
TPG = 4          # tensor-parallel group size (head groups)
DH = E // TPG    # per-core head dims = 256 (4 heads)
NE = E // 128    # 8 contraction tiles
NC5 = 512        # column chunk
NLC = L // NC5   # 4 L-chunks
NST = S // 128   # 16 S tiles
SCALE = 1.0 / np.sqrt(np.float32(D))

_cache = {}


def _build():
    nc = bacc.Bacc("TRN2", target_bir_lowering=False, debug=False, num_devices=1)
    xT = nc.dram_tensor("xT", [E, L], F16, kind="ExternalInput").ap()
    ctxT = nc.dram_tensor("ctxT", [E, S], F16, kind="ExternalInput").ap()
    wq = nc.dram_tensor("wq", [E, DH], F16, kind="ExternalInput").ap()
    wk = nc.dram_tensor("wk", [E, DH], F16, kind="ExternalInput").ap()
    wv = nc.dram_tensor("wv", [E, DH], F16, kind="ExternalInput").ap()
    wp = nc.dram_tensor("wp", [DH, E], F16, kind="ExternalInput").ap()
    bq = nc.dram_tensor("bq", [DH, 1], F32, kind="ExternalInput").ap()
    bk = nc.dram_tensor("bk", [DH, 1], F32, kind="ExternalInput").ap()
    bv = nc.dram_tensor("bv", [1, DH], F16, kind="ExternalInput").ap()
    onesr = nc.dram_tensor("onesr", [1, 128], F16, kind="ExternalInput").ap()
    onesc = nc.dram_tensor("onesc", [128, 1], F16, kind="ExternalInput").ap()
    outp = nc.dram_tensor("outp", [L, E], F32, kind="ExternalOutput").ap()

    with tile.TileContext(nc) as tc:
        with (
            tc.tile_pool(name="wgt", bufs=1) as wgt,
            tc.tile_pool(name="small", bufs=1) as small,
            tc.tile_pool(name="stream", bufs=4) as stream,
            tc.tile_pool(name="qt", bufs=2) as qtp,
            tc.tile_pool(name="kt", bufs=2) as ktp,
            tc.tile_pool(name="vt", bufs=NST) as vtp,
            tc.tile_pool(name="pt", bufs=4) as ptp,
            tc.tile_pool(name="ot", bufs=2) as otp,
            tc.tile_pool(name="ob", bufs=3) as obp,
            tc.tile_pool(name="nrm", bufs=2) as nrm,
            tc.tile_pool(name="gemm", bufs=2, space="PSUM") as gemm,
            tc.tile_pool(name="sc", bufs=2, space="PSUM") as scp,
            tc.tile_pool(name="av", bufs=2, space="PSUM") as avp,
        ):
            # ---- weights / biases / constants ----
            wq_sb = [wgt.tile([128, DH], F16, tag=f"wq{e}", name=f"wq{e}") for e in range(NE)]
            wk_sb = [wgt.tile([128, DH], F16, tag=f"wk{e}", name=f"wk{e}") for e in range(NE)]
            wv_sb = [wgt.tile([128, DH], F16, tag=f"wv{e}", name=f"wv{e}") for e in range(NE)]
            for e in range(NE):
                nc.sync.dma_start(wq_sb[e][:], wq[e * 128:(e + 1) * 128, :])
                nc.sync.dma_start(wk_sb[e][:], wk[e * 128:(e + 1) * 128, :])
                nc.sync.dma_start(wv_sb[e][:], wv[e * 128:(e + 1) * 128, :])
            wp_sb = [wgt.tile([128, E], F16, tag=f"wp{m}", name=f"wp{m}") for m in range(2)]
            for m in range(2):
                nc.sync.dma_start(wp_sb[m][:], wp[m * 128:(m + 1) * 128, :])
            bq_sb = [small.tile([128, 1], F32, tag=f"bq{m}", name=f"bq{m}") for m in range(2)]
            bk_sb = [small.tile([128, 1], F32, tag=f"bk{m}", name=f"bk{m}") for m in range(2)]
            for m in range(2):
                nc.sync.dma_start(bq_sb[m][:], bq[m * 128:(m + 1) * 128, :])
                nc.sync.dma_start(bk_sb[m][:], bk[m * 128:(m + 1) * 128, :])
            bv_sb = small.tile([1, DH], F16, tag="bv")
            nc.sync.dma_start(bv_sb[:], bv[:])
            ones = small.tile([1, 128], F16, tag="ones")
            nc.sync.dma_start(ones[:], onesr[:])
            ones_col = small.tile([128, 1], F16, tag="ones_col")
            nc.sync.dma_start(ones_col[:], onesc[:])

            # chunked [E, 512] views of xT/ctxT: SBUF [128, NE*512], e-major
            def chunk_ap(src, c):
                v = src[:, c * NC5:(c + 1) * NC5]
                return v.rearrange("(e p) l -> p e l", p=128)

            # ---- QT = Wq.T @ xT, per L-chunk ----
            qt_sb = [qtp.tile([128, L], F16, tag=f"qt{m}", name=f"qt{m}") for m in range(2)]

            def qt_chunk(c):
                xc = stream.tile([128, NE * NC5], F16, tag="stream", name=f"xc{c}")
                nc.sync.dma_start(xc[:].rearrange("p (e l) -> p e l", e=NE), chunk_ap(xT, c))
                for m in range(2):
                    pq = gemm.tile([128, NC5], F32, tag="gemm", name=f"pq{c}{m}")
                    for e in range(NE):
                        nc.tensor.matmul(
                            pq[:],
                            wq_sb[e][:, m * 128:(m + 1) * 128],
                            xc[:, e * NC5:(e + 1) * NC5],
                            start=(e == 0), stop=(e == NE - 1),
                        )
                    nc.vector.tensor_scalar(
                        qt_sb[m][:, c * NC5:(c + 1) * NC5], pq[:],
                        bq_sb[m][:, 0:1], None, ADD,
                    )

            # chunk 0 first so attention l-chunk 0 is unblocked the moment
            # KT/V finish; chunks 1-3 are emitted after KT/V and overlap
            # the ACT-bound attention phase.
            qt_chunk(0)

            # ---- KT / V from ctxT, per S-chunk ----
            kt_sb = [ktp.tile([128, S], F16, tag=f"kt{m}", name=f"kt{m}") for m in range(2)]
            v_sb = [vtp.tile([128, 4 * 65], F16, tag="vt", name=f"vt{s}") for s in range(NST)]
            for c in range(NLC):
                cc = stream.tile([128, NE * NC5], F16, tag="stream")
                nc.sync.dma_start(cc[:].rearrange("p (e l) -> p e l", e=NE), chunk_ap(ctxT, c))
                for m in range(2):
                    pk = gemm.tile([128, NC5], F32, tag="gemm")
                    for e in range(NE):
                        nc.tensor.matmul(
                            pk[:],
                            wk_sb[e][:, m * 128:(m + 1) * 128],
                            cc[:, e * NC5:(e + 1) * NC5],
                            start=(e == 0), stop=(e == NE - 1),
                        )
                    nc.vector.tensor_scalar(
                        kt_sb[m][:, c * NC5:(c + 1) * NC5], pk[:],
                        bk_sb[m][:, 0:1], None, ADD,
                    )
                for si in range(4):
                    s = c * 4 + si
                    pv = gemm.tile([128, DH], F32, tag="gemm")
                    for e in range(NE):
                        nc.tensor.matmul(
                            pv[:],
                            cc[:, e * NC5 + si * 128: e * NC5 + (si + 1) * 128],
                            wv_sb[e][:],
                            start=(e == 0), stop=False,
                        )
                    nc.tensor.matmul(
                        pv[:], ones[0:1, :], bv_sb[:], start=False, stop=True,
                    )
                    vt = v_sb[s]
                    for h in range(4):
                        nc.vector.tensor_copy(
                            vt[:, h * 65:h * 65 + 64],
                            pv[:, h * 64:(h + 1) * 64],
                        )
                        nc.vector.tensor_copy(
                            vt[:, h * 65 + 64:h * 65 + 65], ones_col[:],
                        )

            # ---- attention + projection, per L-chunk ----
            # QT for chunk c+1 and projection of chunk c-1 are emitted one
            # matmul per attention s-iteration, so they sit at interleaved
            # scheduler priority and only soak up PE gaps while ACT (exp)
            # is the bottleneck.
            ot_sb = [otp.tile([128, L], F16, tag=f"ot{m}", name=f"ot{m}") for m in range(2)]

            def proj_piece(pc, s, st):
                lt = pc * 4 + s // 4
                nch = (s % 4) // 2
                m = s % 2
                if m == 0:
                    if nch == 0:
                        st["ob"] = obp.tile([128, E], F32, tag="ob",
                                            name=f"ob{pc}_{lt}")
                    st["pp"] = gemm.tile([128, NC5], F32, tag="gemm",
                                         name=f"pp{pc}_{s}")
                nc.tensor.matmul(
                    st["pp"][:],
                    ot_sb[m][:, lt * 128:(lt + 1) * 128],
                    wp_sb[m][:, nch * NC5:(nch + 1) * NC5],
                    start=(m == 0), stop=(m == 1),
                )
                if m == 1:
                    nc.vector.tensor_copy(
                        st["ob"][:, nch * NC5:(nch + 1) * NC5], st["pp"][:],
                    )
                    if nch == 1:
                        nc.sync.dma_start(
                            outp[lt * 128:(lt + 1) * 128, :], st["ob"][:],
                        )

            def qt_piece(c, s, st):
                m, e = s // 8, s % 8
                if e == 0:
                    st["pq"] = gemm.tile([128, NC5], F32, tag="gemm",
                                         name=f"pq{c}_{m}")
                nc.tensor.matmul(
                    st["pq"][:],
                    wq_sb[e][:, m * 128:(m + 1) * 128],
                    st["xc"][:, e * NC5:(e + 1) * NC5],
                    start=(e == 0), stop=(e == NE - 1),
                )
                if e == NE - 1:
                    nc.vector.tensor_scalar(
                        qt_sb[m][:, c * NC5:(c + 1) * NC5], st["pq"][:],
                        bq_sb[m][:, 0:1], None, ADD,
                    )

            for c in range(NLC):
                for hp in range(2):
                    st = {}
                    if hp == 1 and c + 1 < NLC:
                        st["xc"] = stream.tile([128, NE * NC5], F16,
                                               tag="stream", name=f"xc{c+1}")
                        nc.sync.dma_start(
                            st["xc"][:].rearrange("p (e l) -> p e l", e=NE),
                            chunk_ap(xT, c + 1),
                        )
                    av = [avp.tile([65, NC5], F32, tag="av", name=f"av{j}") for j in range(2)]
                    for s in range(NST):
                        sc = scp.tile([128, 2 * NC5], F32, tag="sc")
                        for j in range(2):
                            nc.tensor.matmul(
                                sc[:, j * NC5:(j + 1) * NC5],
                                kt_sb[hp][j * 64:(j + 1) * 64,
                                          s * 128:(s + 1) * 128],
                                qt_sb[hp][j * 64:(j + 1) * 64,
                                          c * NC5:(c + 1) * NC5],
                                start=True, stop=True,
                            )
                        pt = ptp.tile([128, 2 * NC5], F16, tag="pt")
                        nc.scalar.activation(pt[:], sc[:], EXP, scale=float(SCALE))
                        for j in range(2):
                            nc.tensor.matmul(
                                av[j][:],
                                v_sb[s][:, (hp * 2 + j) * 65:(hp * 2 + j + 1) * 65],
                                pt[:, j * NC5:(j + 1) * NC5],
                                start=(s == 0), stop=(s == NST - 1),
                            )
                        if hp == 0 and c >= 1:
                            proj_piece(c - 1, s, st)
                        if hp == 1 and c + 1 < NLC:
                            qt_piece(c + 1, s, st)
                    for j in range(2):
                        # drain the AV psum bank to SBUF right away so the
                        # next group's AV accumulation can reuse it; the rest
                        # of the normalization runs off SBUF.
                        ov = nrm.tile([65, NC5], F32, tag="ov")
                        nc.vector.tensor_copy(ov[:], av[j][:])
                        sums = nrm.tile([1, NC5], F16, tag="sums")
                        nc.vector.tensor_copy(sums[:], ov[64:65, :])
                        rbp = gemm.tile([64, NC5], F32, tag="gemm")
                        nc.tensor.matmul(
                            rbp[:], ones[0:1, 0:64], sums[:], start=True, stop=True,
                        )
                        rb = nrm.tile([64, NC5], F32, tag="rb")
                        nc.vector.reciprocal_approx_fast(out=rb[:], in_=rbp[:])
                        nc.vector.tensor_tensor(
                            ot_sb[hp][j * 64:(j + 1) * 64, c * NC5:(c + 1) * NC5],
                            ov[0:64, :], rb[:], MULT,
                        )
            # tail: project the last L-chunk
            st_tail = {}
            for s in range(NST):
                proj_piece(NLC - 1, s, st_tail)

    nc.compile()
    return nc


def _get_nc():
    if "nc" not in _cache:
        _cache["nc"] = _build()
    return _cache["nc"]


def kernel(x, context, Wq, bq, Wk, bk, Wv, bv, Wp, bp):
    x = np.asarray(x, dtype=np.float32)
    context = np.asarray(context, dtype=np.float32)
    Wq, Wk, Wv, Wp = (np.asarray(a, dtype=np.float32) for a in (Wq, Wk, Wv, Wp))
    bq, bk, bv, bp = (np.asarray(a, dtype=np.float32) for a in (bq, bk, bv, bp))

    nc = _get_nc()
    in_maps = []
    for c in range(NCORES):
        b, g = divmod(c, TPG)
        sl = slice(g * DH, (g + 1) * DH)
        f16 = np.float16
        in_maps.append({
            "xT": np.ascontiguousarray(x[b].T).astype(f16),
            "ctxT": np.ascontiguousarray(context[b].T).astype(f16),
            "wq": np.ascontiguousarray(Wq[:, sl]).astype(f16),
            "wk": np.ascontiguousarray(Wk[:, sl]).astype(f16),
            "wv": np.ascontiguousarray(Wv[:, sl]).astype(f16),
            "wp": np.ascontiguousarray(Wp[sl, :]).astype(f16),
            "bq": np.ascontiguousarray(bq[sl].reshape(DH, 1)),
            "bk": np.ascontiguousarray(bk[sl].reshape(DH, 1)),
            "bv": np.ascontiguousarray(bv[sl].reshape(1, DH)).astype(f16),
            "onesr": np.ones((1, 128), dtype=f16),
            "onesc": np.ones((128, 1), dtype=f16),
        })

    trace = bool(int(__import__("os").environ.get("KERNEL_TRACE", "0")))
    res = run_bass_kernel_spmd(nc, in_maps, list(range(NCORES)), trace=trace)
    _cache["last_results"] = res

    out = np.zeros((B, L, E), dtype=np.float32)
    for c in range(NCORES):
        b = c // TPG
        out[b] += res.results[c]["outp"]
    out += bp.reshape(1, 1, E)
    return out


# revision 10
# speedup vs baseline: 1.4915x; 1.0181x over previous
"""Cross-attention Trainium2 kernel, 8-core SPMD.

Problem: B=2, L=S=2048, E=1024, H=16 heads of D=64.
  q = x@Wq+bq; k = ctx@Wk+bk; v = ctx@Wv+bv  (per-head split)
  out = softmax(q k^T / sqrt(D)) v, heads concat, @Wp + bp

Sharding: DP over batch (2) x TP over heads (4 groups of 4 heads).
Core c: batch b=c//4, head group g=c%4 (heads 4g..4g+3).
Each core computes a partial projection output [L, E]; the host sums the 4
partials per batch and adds bp (Megatron-style TP reduce done host-side).

On-device dataflow (per core), everything transposed so no on-chip
transposes are ever needed:
  QT[dh, L] = Wq_c.T @ xT       (xT, ctxT fed pre-transposed from host)
  KT[dh, S] = Wk_c.T @ ctxT
  V[S, dh]  = (ctxT tiles).T @ Wv_c   (+ ones column per head for row-sums)
  per head pair (row-tiled K=64 matmul pair), per 512-col chunk of L:
    scoresT[s_tile, lc] = KT_tile.T @ QT_chunk        (PSUM [128,1024], 2 heads)
    PT = exp(0.125 * scoresT)                          (one ACT op, fp32r out)
    OT'[65, lc] += (V|1).T @ PT                        (accumulated over s)
  normalize: sums row broadcast via ones-outer-product matmul, DVE recip+mul
  outP[L, E] partial = OT.T @ Wp_c                     (PSUM accum over dh)

All matmul operands are float16 (full 1-cycle/row PE rate; fp32r lowers to
2-cycle/row fp32-HIGH mode on this compiler); PSUM accumulation is fp32.
"""
import sys

if "/opt/trn_rl_repo" not in sys.path:
    sys.path.insert(0, "/opt/trn_rl_repo")

import numpy as np

import concourse.bacc as bacc
import concourse.tile as tile
import concourse.mybir as mybir
from concourse.bass_utils import run_bass_kernel_spmd

F32 = mybir.dt.float32
F16 = mybir.dt.float16
EXP = mybir.ActivationFunctionType.Exp
ADD = mybir.AluOpType.add
MULT = mybir.AluOpType.mult

B, L, S, E, H, D = 2, 2048, 2048, 1024, 16, 64
NCORES = 8
TPG = 4          # tensor-parallel group size (head groups)
DH = E // TPG    # per-core head dims = 256 (4 heads)
NE = E // 128    # 8 contraction tiles
NC5 = 512        # column chunk
NLC = L // NC5   # 4 L-chunks
NST = S // 128   # 16 S tiles
SCALE = 1.0 / np.sqrt(np.float32(D))

_cache = {}


def _build():
    nc = bacc.Bacc("TRN2", target_bir_lowering=False, debug=False, num_devices=1)
    xT = nc.dram_tensor("xT", [E, L], F16, kind="ExternalInput").ap()
    ctxT = nc.dram_tensor("ctxT", [E, S], F16, kind="ExternalInput").ap()
    wq = nc.dram_tensor("wq", [E, DH], F16, kind="ExternalInput").ap()
    wk = nc.dram_tensor("wk", [E, DH], F16, kind="ExternalInput").ap()
    wv = nc.dram_tensor("wv", [E, DH], F16, kind="ExternalInput").ap()
    wp = nc.dram_tensor("wp", [DH, E], F16, kind="ExternalInput").ap()
    bq = nc.dram_tensor("bq", [DH, 1], F32, kind="ExternalInput").ap()
    bk = nc.dram_tensor("bk", [DH, 1], F32, kind="ExternalInput").ap()
    bv = nc.dram_tensor("bv", [1, DH], F16, kind="ExternalInput").ap()
    onesr = nc.dram_tensor("onesr", [1, 128], F16, kind="ExternalInput").ap()
    onesc = nc.dram_tensor("onesc", [128, 1], F16, kind="ExternalInput").ap()
    outp = nc.dram_tensor("outp", [L, E], F32, kind="ExternalOutput").ap()

    with tile.TileContext(nc) as tc:
        with (
            tc.tile_pool(name="wgt", bufs=1) as wgt,
            tc.tile_pool(name="small", bufs=1) as small,
            tc.tile_pool(name="stream", bufs=4) as stream,
            tc.tile_pool(name="qt", bufs=2) as qtp,
            tc.tile_pool(name="kt", bufs=2) as ktp,
            tc.tile_pool(name="vt", bufs=NST) as vtp,
            tc.tile_pool(name="pt", bufs=4) as ptp,
            tc.tile_pool(name="ot", bufs=2) as otp,
            tc.tile_pool(name="ob", bufs=3) as obp,
            tc.tile_pool(name="nrm", bufs=2) as nrm,
            tc.tile_pool(name="gemm", bufs=2, space="PSUM") as gemm,
            tc.tile_pool(name="sc", bufs=2, space="PSUM") as scp,
            tc.tile_pool(name="av", bufs=2, space="PSUM") as avp,
        ):
            # ---- weights / biases / constants ----
            wq_sb = [wgt.tile([128, DH], F16, tag=f"wq{e}", name=f"wq{e}") for e in range(NE)]
            wk_sb = [wgt.tile([128, DH], F16, tag=f"wk{e}", name=f"wk{e}") for e in range(NE)]
            wv_sb = [wgt.tile([128, DH], F16, tag=f"wv{e}", name=f"wv{e}") for e in range(NE)]
            for e in range(NE):
                nc.sync.dma_start(wq_sb[e][:], wq[e * 128:(e + 1) * 128, :])
                nc.sync.dma_start(wk_sb[e][:], wk[e * 128:(e + 1) * 128, :])
                nc.sync.dma_start(wv_sb[e][:], wv[e * 128:(e + 1) * 128, :])
            wp_sb = [wgt.tile([128, E], F16, tag=f"wp{m}", name=f"wp{m}") for m in range(2)]
            for m in range(2):
                nc.sync.dma_start(wp_sb[m][:], wp[m * 128:(m + 1) * 128, :])
            bq_sb = [small.tile([128, 1], F32, tag=f"bq{m}", name=f"bq{m}") for m in range(2)]
            bk_sb = [small.tile([128, 1], F32, tag=f"bk{m}", name=f"bk{m}") for m in range(2)]
            for m in range(2):
                nc.sync.dma_start(bq_sb[m][:], bq[m * 128:(m + 1) * 128, :])
                nc.sync.dma_start(bk_sb[m][:], bk[m * 128:(m + 1) * 128, :])
            bv_sb = small.tile([1, DH], F16, tag="bv")
            nc.sync.dma_start(bv_sb[:], bv[:])
            ones = small.tile([1, 128], F16, tag="ones")
            nc.sync.dma_start(ones[:], onesr[:])
            ones_col = small.tile([128, 1], F16, tag="ones_col")
            nc.sync.dma_start(ones_col[:], onesc[:])

            # chunked [E, 512] views of xT/ctxT: SBUF [128, NE*512], e-major
            def chunk_ap(src, c):
                v = src[:, c * NC5:(c + 1) * NC5]
                return v.rearrange("(e p) l -> p e l", p=128)

            # ---- QT = Wq.T @ xT, per L-chunk ----
            qt_sb = [qtp.tile([128, L], F16, tag=f"qt{m}", name=f"qt{m}") for m in range(2)]

            def qt_chunk(c):
                xc = stream.tile([128, NE * NC5], F16, tag="stream", name=f"xc{c}")
                nc.sync.dma_start(xc[:].rearrange("p (e l) -> p e l", e=NE), chunk_ap(xT, c))
                for m in range(2):
                    pq = gemm.tile([128, NC5], F32, tag="gemm", name=f"pq{c}{m}")
                    for e in range(NE):
                        nc.tensor.matmul(
                            pq[:],
                            wq_sb[e][:, m * 128:(m + 1) * 128],
                            xc[:, e * NC5:(e + 1) * NC5],
                            start=(e == 0), stop=(e == NE - 1),
                        )
                    nc.vector.tensor_scalar(
                        qt_sb[m][:, c * NC5:(c + 1) * NC5], pq[:],
                        bq_sb[m][:, 0:1], None, ADD,
                    )

            # chunk 0 first so attention l-chunk 0 is unblocked the moment
            # KT/V finish; chunks 1-3 are emitted after KT/V and overlap
            # the ACT-bound attention phase.
            qt_chunk(0)

            # ---- KT / V from ctxT, per S-chunk ----
            kt_sb = [ktp.tile([128, S], F16, tag=f"kt{m}", name=f"kt{m}") for m in range(2)]
            v_sb = [vtp.tile([128, 4 * 65], F16, tag="vt", name=f"vt{s}") for s in range(NST)]

            def attn_iter(lc, hp, s, av):
                sc = scp.tile([128, 2 * NC5], F32, tag="sc", name=f"sc{lc}{hp}{s}")
                for j in range(2):
                    nc.tensor.matmul(
                        sc[:, j * NC5:(j + 1) * NC5],
                        kt_sb[hp][j * 64:(j + 1) * 64, s * 128:(s + 1) * 128],
                        qt_sb[hp][j * 64:(j + 1) * 64, lc * NC5:(lc + 1) * NC5],
                        start=True, stop=True,
                    )
                pt = ptp.tile([128, 2 * NC5], F16, tag="pt", name=f"pt{lc}{hp}{s}")
                nc.scalar.activation(pt[:], sc[:], EXP, scale=float(SCALE))
                for j in range(2):
                    nc.tensor.matmul(
                        av[j][:],
                        v_sb[s][:, (hp * 2 + j) * 65:(hp * 2 + j + 1) * 65],
                        pt[:, j * NC5:(j + 1) * NC5],
                        start=(s == 0), stop=(s == NST - 1),
                    )

            # (l-chunk 0, head-pair 0) attention is interleaved into the KT/V
            # chunk loop below so ACT starts exp'ing as soon as the first
            # context chunk's KT and V tiles exist.
            av00 = [avp.tile([65, NC5], F32, tag="av", name=f"av00{j}") for j in range(2)]
            for c in range(NLC):
                cc = stream.tile([128, NE * NC5], F16, tag="stream")
                nc.sync.dma_start(cc[:].rearrange("p (e l) -> p e l", e=NE), chunk_ap(ctxT, c))
                for m in range(2):
                    pk = gemm.tile([128, NC5], F32, tag="gemm")
                    for e in range(NE):
                        nc.tensor.matmul(
                            pk[:],
                            wk_sb[e][:, m * 128:(m + 1) * 128],
                            cc[:, e * NC5:(e + 1) * NC5],
                            start=(e == 0), stop=(e == NE - 1),
                        )
                    nc.vector.tensor_scalar(
                        kt_sb[m][:, c * NC5:(c + 1) * NC5], pk[:],
                        bk_sb[m][:, 0:1], None, ADD,
                    )
                for si in range(4):
                    s = c * 4 + si
                    pv = gemm.tile([128, DH], F32, tag="gemm")
                    for e in range(NE):
                        nc.tensor.matmul(
                            pv[:],
                            cc[:, e * NC5 + si * 128: e * NC5 + (si + 1) * 128],
                            wv_sb[e][:],
                            start=(e == 0), stop=False,
                        )
                    nc.tensor.matmul(
                        pv[:], ones[0:1, :], bv_sb[:], start=False, stop=True,
                    )
                    vt = v_sb[s]
                    for h in range(4):
                        nc.vector.tensor_copy(
                            vt[:, h * 65:h * 65 + 64],
                            pv[:, h * 64:(h + 1) * 64],
                        )
                        nc.vector.tensor_copy(
                            vt[:, h * 65 + 64:h * 65 + 65], ones_col[:],
                        )
                for si in range(4):
                    attn_iter(0, 0, c * 4 + si, av00)

            # ---- attention + projection, per L-chunk ----
            # QT for chunk c+1 and projection of chunk c-1 are emitted one
            # matmul per attention s-iteration, so they sit at interleaved
            # scheduler priority and only soak up PE gaps while ACT (exp)
            # is the bottleneck.
            ot_sb = [otp.tile([128, L], F16, tag=f"ot{m}", name=f"ot{m}") for m in range(2)]

            def proj_piece(pc, s, st):
                lt = pc * 4 + s // 4
                nch = (s % 4) // 2
                m = s % 2
                if m == 0:
                    if nch == 0:
                        st["ob"] = obp.tile([128, E], F32, tag="ob",
                                            name=f"ob{pc}_{lt}")
                    st["pp"] = gemm.tile([128, NC5], F32, tag="gemm",
                                         name=f"pp{pc}_{s}")
                nc.tensor.matmul(
                    st["pp"][:],
                    ot_sb[m][:, lt * 128:(lt + 1) * 128],
                    wp_sb[m][:, nch * NC5:(nch + 1) * NC5],
                    start=(m == 0), stop=(m == 1),
                )
                if m == 1:
                    nc.vector.tensor_copy(
                        st["ob"][:, nch * NC5:(nch + 1) * NC5], st["pp"][:],
                    )
                    if nch == 1:
                        nc.sync.dma_start(
                            outp[lt * 128:(lt + 1) * 128, :], st["ob"][:],
                        )

            def qt_piece(c, s, st):
                m, e = s // 8, s % 8
                if e == 0:
                    st["pq"] = gemm.tile([128, NC5], F32, tag="gemm",
                                         name=f"pq{c}_{m}")
                nc.tensor.matmul(
                    st["pq"][:],
                    wq_sb[e][:, m * 128:(m + 1) * 128],
                    st["xc"][:, e * NC5:(e + 1) * NC5],
                    start=(e == 0), stop=(e == NE - 1),
                )
                if e == NE - 1:
                    nc.vector.tensor_scalar(
                        qt_sb[m][:, c * NC5:(c + 1) * NC5], st["pq"][:],
                        bq_sb[m][:, 0:1], None, ADD,
                    )

            def norm_group(lc, hp, av):
                for j in range(2):
                    # drain the AV psum bank to SBUF right away so the next
                    # group's AV accumulation can reuse it; the rest of the
                    # normalization runs off SBUF.
                    ov = nrm.tile([65, NC5], F32, tag="ov")
                    nc.vector.tensor_copy(ov[:], av[j][:])
                    sums = nrm.tile([1, NC5], F16, tag="sums")
                    nc.vector.tensor_copy(sums[:], ov[64:65, :])
                    rbp = gemm.tile([64, NC5], F32, tag="gemm")
                    nc.tensor.matmul(
                        rbp[:], ones[0:1, 0:64], sums[:], start=True, stop=True,
                    )
                    rb = nrm.tile([64, NC5], F32, tag="rb")
                    nc.vector.reciprocal_approx_fast(out=rb[:], in_=rbp[:])
                    nc.vector.tensor_tensor(
                        ot_sb[hp][j * 64:(j + 1) * 64, lc * NC5:(lc + 1) * NC5],
                        ov[0:64, :], rb[:], MULT,
                    )

            for c in range(NLC):
                for hp in range(2):
                    if c == 0 and hp == 0:
                        norm_group(0, 0, av00)
                        continue
                    st = {}
                    if hp == 1 and c + 1 < NLC:
                        st["xc"] = stream.tile([128, NE * NC5], F16,
                                               tag="stream", name=f"xc{c+1}")
                        nc.sync.dma_start(
                            st["xc"][:].rearrange("p (e l) -> p e l", e=NE),
                            chunk_ap(xT, c + 1),
                        )
                    av = [avp.tile([65, NC5], F32, tag="av", name=f"av{c}{hp}{j}") for j in range(2)]
                    for s in range(NST):
                        attn_iter(c, hp, s, av)
                        if hp == 0 and c >= 1:
                            proj_piece(c - 1, s, st)
                        if hp == 1 and c + 1 < NLC:
                            qt_piece(c + 1, s, st)
                    norm_group(c, hp, av)
            # tail: project the last L-chunk
            st_tail = {}
            for s in range(NST):
                proj_piece(NLC - 1, s, st_tail)

    nc.compile()
    return nc


def _get_nc():
    if "nc" not in _cache:
        _cache["nc"] = _build()
    return _cache["nc"]


def kernel(x, context, Wq, bq, Wk, bk, Wv, bv, Wp, bp):
    x = np.asarray(x, dtype=np.float32)
    context = np.asarray(context, dtype=np.float32)
    Wq, Wk, Wv, Wp = (np.asarray(a, dtype=np.float32) for a in (Wq, Wk, Wv, Wp))
    bq, bk, bv, bp = (np.asarray(a, dtype=np.float32) for a in (bq, bk, bv, bp))

    nc = _get_nc()
    in_maps = []
    for c in range(NCORES):
        b, g = divmod(c, TPG)
        sl = slice(g * DH, (g + 1) * DH)
        f16 = np.float16
        in_maps.append({
            "xT": np.ascontiguousarray(x[b].T).astype(f16),
            "ctxT": np.ascontiguousarray(context[b].T).astype(f16),
            "wq": np.ascontiguousarray(Wq[:, sl]).astype(f16),
            "wk": np.ascontiguousarray(Wk[:, sl]).astype(f16),
            "wv": np.ascontiguousarray(Wv[:, sl]).astype(f16),
            "wp": np.ascontiguousarray(Wp[sl, :]).astype(f16),
            "bq": np.ascontiguousarray(bq[sl].reshape(DH, 1)),
            "bk": np.ascontiguousarray(bk[sl].reshape(DH, 1)),
            "bv": np.ascontiguousarray(bv[sl].reshape(1, DH)).astype(f16),
            "onesr": np.ones((1, 128), dtype=f16),
            "onesc": np.ones((128, 1), dtype=f16),
        })

    trace = bool(int(__import__("os").environ.get("KERNEL_TRACE", "0")))
    res = run_bass_kernel_spmd(nc, in_maps, list(range(NCORES)), trace=trace)
    _cache["last_results"] = res

    out = np.zeros((B, L, E), dtype=np.float32)
    for c in range(NCORES):
        b = c // TPG
        out[b] += res.results[c]["outp"]
    out += bp.reshape(1, 1, E)
    return out
